# revision 17
# baseline (speedup 1.0000x reference)
"""MoE (DeepSeek-style) routed+shared expert forward on 8 TRN2 NeuronCores.

Strategy (expert-parallel, host-side dispatch):
  - Host computes the gate (softmax + top-2) in float64 and gathers each
    expert's routed tokens; core e processes expert e's tokens (padded to
    capacity C) plus a 1/8 slice of all tokens through the replicated
    shared-expert MLP.
  - Routed expert runs in fp8(e4m3) with DoubleRow matmuls (2x PE rate).
    Host quantizes x and the expert weights with per-tensor scales; the
    w1-path dequant folds into the SILU activation's scale operand and
    the w3 scale is tied to the sampled h distribution (s3 = sh/sx) so
    the ps3 PSUM result IS h*sh exactly — the h tile is produced by a
    single vector multiply reading PSUM, no dequant op. Error budget:
    the routed path carries only ~23% of the output norm (gate weights
    are softmax scores ~0.2), so fp8's ~6% relative error lands at
    ~1.6e-2 overall, within the 2e-2 tolerance.
  - Shared expert (97% of the output norm) stays bf16.
  - All weights are SBUF-resident; inputs arrive via a handful of large
    DMAs (one per operand) ordered to match PE consumption, with the
    shared-expert weights trickled in across the routed block loop so
    routed output DMAs never sit behind a multi-MB preload backlog.
  - Warmup matmuls on scratch SBUF run during the input DMA so the PE's
    DVFS clock is fully ramped when real work arrives.
"""

import sys

if "/opt/trn_rl_repo" not in sys.path:
    sys.path.insert(0, "/opt/trn_rl_repo")

import ml_dtypes
import numpy as np

import concourse.bass as bass
import concourse.tile as tile
from concourse import bacc, mybir
from concourse import bass_utils
from concourse.alu_op_type import AluOpType

B, S, DIM = 4, 2048, 1024
T = B * S
INTER = 1024
E = 8
TOPK = 2
ROUTE_SCALE = 1.0
SHARED_INTER = 2048
N_CORES = 8
TS = T // N_CORES  # shared-expert tokens per core
BLK = 512
N_WARM = 16

F32 = mybir.dt.float32
BF16 = mybir.dt.bfloat16
FP8 = mybir.dt.float8e4
SILU = mybir.ActivationFunctionType.Silu
IDENT = mybir.ActivationFunctionType.Identity
DR = mybir.MatmulPerfMode.DoubleRow
MUL = AluOpType.mult
ADD = AluOpType.add

E4NP = ml_dtypes.float8_e4m3fn
BFNP = ml_dtypes.bfloat16

ND = DIM // 128           # 8 k-tiles over DIM
NP = ND // 2              # 4 DoubleRow k-pair tiles over DIM
NI = INTER // 128         # 8 tiles over INTER
NS = SHARED_INTER // 128  # 16 tiles over SHARED_INTER

_program_cache = {}


def _blocks(total):
    """Split into <=512-wide even blocks of near-equal size (all >=256 so
    per-instruction LDWEIGHTS overhead stays hidden)."""
    nb = -(-total // BLK)
    b = -(-total // (nb * 32)) * 32
    sizes = [b] * (nb - 1) + [total - b * (nb - 1)]
    assert all(256 <= s <= BLK and s % 2 == 0 for s in sizes), sizes
    out, o = [], 0
    for s in sizes:
        out.append((o, s))
        o += s
    return out


def build_program(C, general):
    nc = bacc.Bacc("TRN2", target_bir_lowering=False, debug=False,
                   num_devices=N_CORES)

    def din(name, shape, dt):
        return nc.dram_tensor(name, shape, dt, kind="ExternalInput").ap()

    def dout(name, shape, dt):
        return nc.dram_tensor(name, shape, dt, kind="ExternalOutput").ap()

    xe8 = din("xe8", (128, ND, C), FP8)        # routed tokens, fp8 pairs
    w18 = din("w18", (128, ND, INTER), FP8)    # w1[e].T in DR pair layout
    w38 = din("w38", (128, ND, INTER), FP8)
    w28 = din("w28", (128, ND, DIM), FP8)      # w2[e].T in DR pair layout
    xs = din("xs", (128, ND, TS), BF16)        # shared-token slice
    ws1 = din("ws1", (128, ND, SHARED_INTER), BF16)
    ws3 = din("ws3", (128, ND, SHARED_INTER), BF16)
    ws2 = din("ws2", (128, NS, DIM), BF16)
    scb = din("scb", (128, 68), F32)  # packed biases + dequant scales
    ye = dout("ye", (ND, 128, C), BF16)
    ys = dout("ys", (ND, 128, TS), BF16)

    rblocks = _blocks(C)
    sblocks = _blocks(TS)

    with tile.TileContext(nc) as tc:
        with tc.tile_pool(name="const", bufs=1) as cpool, \
             tc.tile_pool(name="tmp", bufs=2) as tpool, \
             tc.tile_pool(name="hr", bufs=2) as hrpool, \
             tc.tile_pool(name="hsh", bufs=2) as hspool, \
             tc.tile_pool(name="yout", bufs=6) as ypool, \
             tc.tile_pool(name="ysout", bufs=2) as yspool, \
             tc.tile_pool(name="ps1p", bufs=3, space="PSUM") as ps1pool, \
             tc.tile_pool(name="ps3p", bufs=3, space="PSUM") as ps3pool, \
             tc.tile_pool(name="ps", bufs=2, space="PSUM") as pspool:

            # ---- PE warmup: ramp the DVFS clock while input DMA runs ----
            wsc = cpool.tile([128, 416], BF16, tag="warm")
            nc.vector.memset(wsc[:], 0.25)
            for _ in range(N_WARM):
                wps = pspool.tile([128, 416], F32, tag="psy",
                                  padded_shape=[128, BLK])
                nc.tensor.matmul(wps[:], wsc[:, 0:128], wsc[:],
                                 start=True, stop=True)

            # ---- input DMAs, large transfers in PE consumption order ----
            ball = cpool.tile([128, 68], F32, tag="scb")
            nc.sync.dma_start(ball[:], scb[:])
            b1c = lambda mi: ball[:, mi:mi + 1]
            b3c = lambda mi: ball[:, 8 + mi:9 + mi]
            b2c = lambda md: ball[:, 16 + md:17 + md]
            bs1c = lambda mi: ball[:, 24 + mi:25 + mi]
            bs3c = lambda mi: ball[:, 40 + mi:41 + mi]
            bs2c = lambda md: ball[:, 56 + md:57 + md]
            sc1 = ball[:, 64:65]
            sc3 = ball[:, 65:66]
            scy = ball[:, 66:67]

            hi = INTER // 2
            w1_t = cpool.tile([128, ND, INTER], FP8, tag="w1")
            w3_t = cpool.tile([128, ND, INTER], FP8, tag="w3")
            xe_t = cpool.tile([128, ND, C], FP8, tag="xe", name="xe")
            w2_t = cpool.tile([128, ND, DIM], FP8, tag="w2")
            off0, n0 = rblocks[0]
            nc.sync.dma_start(w1_t[:, :, 0:hi], w18[:, :, 0:hi])
            nc.sync.dma_start(xe_t[:, :, off0:off0 + n0],
                              xe8[:, :, off0:off0 + n0])
            nc.sync.dma_start(w3_t[:, :, 0:hi], w38[:, :, 0:hi])
            nc.sync.dma_start(w1_t[:, :, hi:INTER], w18[:, :, hi:INTER])
            nc.sync.dma_start(w3_t[:, :, hi:INTER], w38[:, :, hi:INTER])
            nc.sync.dma_start(w2_t[:], w28[:])
            for (off, n) in rblocks[1:]:
                nc.sync.dma_start(xe_t[:, :, off:off + n],
                                  xe8[:, :, off:off + n])
            xs_t = cpool.tile([128, ND, TS], BF16, tag="xs", name="xs")
            nc.sync.dma_start(xs_t[:], xs[:])
            ws1_t = cpool.tile([128, ND, SHARED_INTER], BF16, tag="ws1",
                               name="ws1")
            ws3_t = cpool.tile([128, ND, SHARED_INTER], BF16, tag="ws3",
                               name="ws3")
            ws2_t = cpool.tile([128, NS, DIM], BF16, tag="ws2", name="ws2")
            # shared-weight DMAs trickled across the routed block loop in
            # ~2MB chunks so routed output DMAs never queue behind them
            hsi = SHARED_INTER // 2
            ws_dmas = [
                (ws1_t[:, :, 0:hsi], ws1[:, :, 0:hsi]),
                (ws1_t[:, :, hsi:SHARED_INTER], ws1[:, :, hsi:SHARED_INTER]),
                (ws3_t[:, :, 0:hsi], ws3[:, :, 0:hsi]),
                (ws3_t[:, :, hsi:SHARED_INTER], ws3[:, :, hsi:SHARED_INTER]),
                (ws2_t[:, 0:NS // 2, :], ws2[:, 0:NS // 2, :]),
                (ws2_t[:, NS // 2:NS, :], ws2[:, NS // 2:NS, :]),
            ]
            nws = -(-len(ws_dmas) // len(rblocks))

            # ---- Phase 1: routed expert, fp8 DoubleRow ----
            h8s = {}

            def r_h8(bi):
                if bi not in h8s:
                    n = rblocks[bi][1]
                    h8s[bi] = [hrpool.tile([128, 2, n], FP8, tag=f"h8_{j}",
                                           name=f"h8{j}",
                                           padded_shape=[128, 2, BLK])
                               for j in range(NP)]
                return h8s[bi]

            def emit_mi(bi, mi):
                off, n = rblocks[bi]
                h8 = r_h8(bi)
                ps1 = ps1pool.tile([128, n], F32, tag="ps1",
                                   padded_shape=[128, BLK])
                for j in range(NP):
                    nc.tensor.matmul(
                        ps1[:], w1_t[:, 2 * j:2 * j + 2,
                                     mi * 128:(mi + 1) * 128],
                        xe_t[:, 2 * j:2 * j + 2, off:off + n],
                        start=(j == 0), stop=(j == NP - 1), perf_mode=DR)
                ps3 = ps3pool.tile([128, n], F32, tag="ps3",
                                   padded_shape=[128, BLK])
                for j in range(NP):
                    nc.tensor.matmul(
                        ps3[:], w3_t[:, 2 * j:2 * j + 2,
                                     mi * 128:(mi + 1) * 128],
                        xe_t[:, 2 * j:2 * j + 2, off:off + n],
                        start=(j == 0), stop=(j == NP - 1), perf_mode=DR)
                t1 = tpool.tile([128, n], BF16, tag="t1", name="t1",
                                padded_shape=[128, BLK])
                nc.scalar.activation(t1[:], ps1[:], SILU,
                                     bias=b1c(mi), scale=sc1)
                if general:
                    t3 = tpool.tile([128, n], BF16, tag="t3", name="t3",
                                    padded_shape=[128, BLK])
                    nc.vector.tensor_scalar(t3[:], ps3[:], sc3, b3c(mi),
                                            MUL, ADD)
                    nc.vector.tensor_mul(h8[mi // 2][:, mi % 2, :],
                                         t1[:], t3[:])
                else:
                    # s3 is chosen so ps3 == h-precursor * sh exactly
                    nc.vector.tensor_mul(h8[mi // 2][:, mi % 2, :],
                                         t1[:], ps3[:])

            for bi, (off, n) in enumerate(rblocks):
                for dst, src in ws_dmas[bi * nws:(bi + 1) * nws]:
                    nc.sync.dma_start(dst, src)
                for mi in (range(NI) if bi == 0 else range(2, NI)):
                    emit_mi(bi, mi)
                if bi + 1 < len(rblocks):
                    emit_mi(bi + 1, 0)  # lookahead covers h8 vector lag
                    emit_mi(bi + 1, 1)
                h8 = h8s.pop(bi)
                for md in range(ND):
                    psy = pspool.tile([128, n], F32, tag="psy",
                                      padded_shape=[128, BLK])
                    for j in range(NP):
                        nc.tensor.matmul(
                            psy[:], w2_t[:, 2 * j:2 * j + 2,
                                         md * 128:(md + 1) * 128],
                            h8[j][:, :, :],
                            start=(j == 0), stop=(j == NP - 1),
                            perf_mode=DR)
                    yt = ypool.tile([128, n], BF16, tag="yt", name="yt",
                                    padded_shape=[128, BLK])
                    nc.scalar.activation(yt[:], psy[:], IDENT,
                                         bias=b2c(md), scale=scy)
                    nc.sync.dma_start(ye[md][:, off:off + n], yt[:])

            # ---- Phase 2: shared expert, bf16 ----
            for (off, n) in sblocks:
                hs = [hspool.tile([128, n], BF16, tag=f"hs_{mi}",
                                  name=f"hs{mi}", padded_shape=[128, BLK])
                      for mi in range(NS)]
                for mi in range(NS):
                    ps1 = ps1pool.tile([128, n], F32, tag="ps1",
                                       padded_shape=[128, BLK])
                    for dk in range(ND):
                        nc.tensor.matmul(
                            ps1[:], ws1_t[:, dk, mi * 128:(mi + 1) * 128],
                            xs_t[:, dk, off:off + n],
                            start=(dk == 0), stop=(dk == ND - 1))
                    ps3 = ps3pool.tile([128, n], F32, tag="ps3",
                                       padded_shape=[128, BLK])
                    for dk in range(ND):
                        nc.tensor.matmul(
                            ps3[:], ws3_t[:, dk, mi * 128:(mi + 1) * 128],
                            xs_t[:, dk, off:off + n],
                            start=(dk == 0), stop=(dk == ND - 1))
                    t1 = tpool.tile([128, n], BF16, tag="t1", name="t1",
                                    padded_shape=[128, BLK])
                    nc.scalar.activation(t1[:], ps1[:], SILU, bias=bs1c(mi))
                    if general:
                        t3 = tpool.tile([128, n], BF16, tag="t3", name="t3",
                                        padded_shape=[128, BLK])
                        nc.vector.tensor_scalar(t3[:], ps3[:], 1.0, bs3c(mi),
                                                MUL, ADD)
                        nc.vector.tensor_mul(hs[mi][:], t1[:], t3[:])
                    else:
                        nc.vector.tensor_mul(hs[mi][:], t1[:], ps3[:])
                for md in range(ND):
                    psy = pspool.tile([128, n], F32, tag="psy",
                                      padded_shape=[128, BLK])
                    for mi in range(NS):
                        nc.tensor.matmul(
                            psy[:], ws2_t[:, mi, md * 128:(md + 1) * 128],
                            hs[mi][:],
                            start=(mi == 0), stop=(mi == NS - 1))
                    yts = yspool.tile([128, n], BF16, tag="yts", name="yts",
                                      padded_shape=[128, BLK])
                    nc.scalar.activation(yts[:], psy[:], IDENT, bias=bs2c(md))
                    nc.sync.dma_start(ys[md][:, off:off + n], yts[:])

    nc.compile()
    return nc


def _q8(a):
    return np.clip(a, -448.0, 448.0).astype(E4NP)


def _pack_w(w, scale):
    """[out, K] weight -> [128, K/128, out] fp8 DoubleRow pair layout
    (partition-major; k-subtile pairs adjacent in the middle dim)."""
    K = w.shape[1]
    A = (w.T * scale).reshape(K // 256, 2, 128, w.shape[0])
    return _q8(np.ascontiguousarray(
        A.transpose(2, 0, 1, 3).reshape(128, K // 128, w.shape[0])))


def _pack_x(xg, scale, C):
    """[n, DIM] tokens -> [128, DIM/128, C] fp8 DoubleRow pair layout."""
    A = np.zeros((DIM, C), np.float32)
    A[:, :xg.shape[0]] = (xg * scale).T
    A = A.reshape(NP, 2, 128, C)
    return _q8(np.ascontiguousarray(
        A.transpose(2, 0, 1, 3).reshape(128, ND, C)))


def _pack_bf(w_t, nk):
    """[K, M] (already transposed) -> [128, nk, M] bf16."""
    K, M = w_t.shape
    return np.ascontiguousarray(
        w_t.reshape(nk, 128, M).transpose(1, 0, 2)).astype(BFNP)


def _gate_host(xt, gate_w, gate_b):
    logits = xt.astype(np.float64) @ gate_w.astype(np.float64).T \
        + gate_b.astype(np.float64)
    m = logits.max(axis=-1, keepdims=True)
    p = np.exp(logits - m)
    scores = p / p.sum(axis=-1, keepdims=True)
    order = np.argsort(-scores, axis=1, kind="stable")
    top_i = order[:, :TOPK]
    top_w = (np.take_along_axis(scores, top_i, axis=1)
             * ROUTE_SCALE).astype(np.float32)
    return top_i, top_w


def run(inputs, trace=False):
    x = np.ascontiguousarray(np.asarray(inputs["x"], dtype=np.float32))
    gate_w = np.asarray(inputs["gate_w"], dtype=np.float32)
    gate_b = np.asarray(inputs["gate_b"], dtype=np.float32)
    w1 = np.asarray(inputs["w1"], dtype=np.float32)
    b1 = np.asarray(inputs["b1"], dtype=np.float32)
    w3 = np.asarray(inputs["w3"], dtype=np.float32)
    b3 = np.asarray(inputs["b3"], dtype=np.float32)
    w2 = np.asarray(inputs["w2"], dtype=np.float32)
    b2 = np.asarray(inputs["b2"], dtype=np.float32)
    ws1 = np.asarray(inputs["ws1"], dtype=np.float32)
    bs1 = np.asarray(inputs["bs1"], dtype=np.float32)
    ws3 = np.asarray(inputs["ws3"], dtype=np.float32)
    bs3 = np.asarray(inputs["bs3"], dtype=np.float32)
    ws2 = np.asarray(inputs["ws2"], dtype=np.float32)
    bs2 = np.asarray(inputs["bs2"], dtype=np.float32)

    xt = x.reshape(T, DIM)
    top_i, top_w = _gate_host(xt, gate_w, gate_b)

    idx, wgt = [], []
    for e in range(E):
        toks = np.nonzero((top_i == e).any(axis=1))[0]
        idx.append(toks)
        slot = (top_i[toks] == e)
        wgt.append(top_w[toks][slot])

    cmax = max(len(i) for i in idx)
    C = max(256, -(-cmax // 32) * 32)

    # fp8 scales: per-tensor for x, per-expert per-tensor for weights; the
    # h scale comes from a 32-token fp32 sample of the true h distribution.
    sx = 8.0 / max(xt.std(), 1e-30)
    xprobe = xt[:32]
    s1 = np.empty(E, np.float64); s3 = np.empty(E, np.float64)
    s2 = np.empty(E, np.float64); sh = np.empty(E, np.float64)
    for e in range(E):
        s1[e] = 16.0 / max(w1[e].std(), 1e-30)
        s2[e] = 16.0 / max(w2[e].std(), 1e-30)
        a = xprobe @ w1[e].T + b1[e]
        bb = xprobe @ w3[e].T + b3[e]
        h = a / (1.0 + np.exp(-a)) * bb
        # tie s3 to the h scale so ps3 needs no dequant (sh == sx*s3)
        s3[e] = 8.0 / max(h.std(), 1e-30) / sx
        sh[e] = sx * s3[e]

    ws1p = _pack_bf(ws1.T, ND)
    ws3p = _pack_bf(ws3.T, ND)
    ws2p = _pack_bf(ws2.T, NS)

    in_maps = []
    for e in range(E):
        scbuf = np.zeros((128, 68), np.float32)
        scbuf[:, 0:8] = b1[e].reshape(8, 128).T
        scbuf[:, 8:16] = (b3[e] * sh[e]).reshape(8, 128).T
        scbuf[:, 16:24] = b2[e].reshape(8, 128).T
        scbuf[:, 24:40] = bs1.reshape(16, 128).T
        scbuf[:, 40:56] = bs3.reshape(16, 128).T
        scbuf[:, 56:64] = bs2.reshape(8, 128).T
        scbuf[:, 64] = 1.0 / (sx * s1[e])
        scbuf[:, 65] = sh[e] / (sx * s3[e])
        scbuf[:, 66] = 1.0 / (sh[e] * s2[e])
        sl = slice(TS * e, TS * (e + 1))
        in_maps.append({
            "xe8": _pack_x(xt[idx[e]], sx, C),
            "w18": _pack_w(w1[e], s1[e]),
            "w38": _pack_w(w3[e], s3[e]),
            "w28": _pack_w(w2[e], s2[e]),
            "xs": _pack_bf(xt[sl].T, ND),
            "ws1": ws1p, "ws3": ws3p, "ws2": ws2p,
            "scb": scbuf,
        })

    general = bool(np.any(b3) or np.any(bs3))
    key = (C, general)
    if key not in _program_cache:
        _program_cache[key] = build_program(C, general)
    nc = _program_cache[key]

    res = bass_utils.run_bass_kernel_spmd(
        nc, in_maps, core_ids=list(range(N_CORES)), trace=trace)

    y = np.empty((T, DIM), np.float32)
    for e in range(E):
        sl = slice(TS * e, TS * (e + 1))
        y[sl] = res.results[e]["ys"].reshape(DIM, TS).T.astype(np.float32)
    for e in range(E):
        yee = res.results[e]["ye"].reshape(DIM, C).astype(np.float32)
        y[idx[e]] += yee[:, :len(idx[e])].T * wgt[e][:, None]
    return y.reshape(B, S, DIM), res


def kernel(**inputs) -> np.ndarray:
    out, _ = run(inputs, trace=False)
    return out


# revision 18
# speedup vs baseline: 1.0065x; 1.0065x over previous
"""MoE (DeepSeek-style) routed+shared expert forward on 8 TRN2 NeuronCores.

Strategy (expert-parallel, host-side dispatch):
  - Host computes the gate (softmax + top-2) in float64 and gathers each
    expert's routed tokens; core e processes expert e's tokens (padded to
    capacity C) plus a 1/8 slice of all tokens through the replicated
    shared-expert MLP.
  - Routed expert runs in fp8(e4m3) with DoubleRow matmuls (2x PE rate).
    Host quantizes x and the expert weights with per-tensor scales; the
    w1-path dequant folds into the SILU activation's scale operand and
    the w3 scale is tied to the sampled h distribution (s3 = sh/sx) so
    the ps3 PSUM result IS h*sh exactly — the h tile is produced by a
    single vector multiply reading PSUM, no dequant op. Error budget:
    the routed path carries only ~23% of the output norm (gate weights
    are softmax scores ~0.2), so fp8's ~6% relative error lands at
    ~1.6e-2 overall, within the 2e-2 tolerance.
  - Shared expert (97% of the output norm) stays bf16.
  - All weights are SBUF-resident; inputs arrive via a handful of large
    DMAs (one per operand) ordered to match PE consumption, with the
    shared-expert weights trickled in across the routed block loop so
    routed output DMAs never sit behind a multi-MB preload backlog.
  - Warmup matmuls on scratch SBUF run during the input DMA so the PE's
    DVFS clock is fully ramped when real work arrives.
"""

import sys

if "/opt/trn_rl_repo" not in sys.path:
    sys.path.insert(0, "/opt/trn_rl_repo")

import ml_dtypes
import numpy as np

import concourse.bass as bass
import concourse.tile as tile
from concourse import bacc, mybir
from concourse import bass_utils
from concourse.alu_op_type import AluOpType

B, S, DIM = 4, 2048, 1024
T = B * S
INTER = 1024
E = 8
TOPK = 2
ROUTE_SCALE = 1.0
SHARED_INTER = 2048
N_CORES = 8
TS = T // N_CORES  # shared-expert tokens per core
BLK = 512
N_WARM = 20

F32 = mybir.dt.float32
BF16 = mybir.dt.bfloat16
FP8 = mybir.dt.float8e4
SILU = mybir.ActivationFunctionType.Silu
IDENT = mybir.ActivationFunctionType.Identity
DR = mybir.MatmulPerfMode.DoubleRow
MUL = AluOpType.mult
ADD = AluOpType.add

E4NP = ml_dtypes.float8_e4m3fn
BFNP = ml_dtypes.bfloat16

ND = DIM // 128           # 8 k-tiles over DIM
NP = ND // 2              # 4 DoubleRow k-pair tiles over DIM
NI = INTER // 128         # 8 tiles over INTER
NS = SHARED_INTER // 128  # 16 tiles over SHARED_INTER

_program_cache = {}


def _blocks(total):
    """Split into <=512-wide even blocks of near-equal size (all >=256 so
    per-instruction LDWEIGHTS overhead stays hidden)."""
    nb = -(-total // BLK)
    b = -(-total // (nb * 32)) * 32
    sizes = [b] * (nb - 1) + [total - b * (nb - 1)]
    assert all(256 <= s <= BLK and s % 2 == 0 for s in sizes), sizes
    out, o = [], 0
    for s in sizes:
        out.append((o, s))
        o += s
    return out


def build_program(C, general):
    nc = bacc.Bacc("TRN2", target_bir_lowering=False, debug=False,
                   num_devices=N_CORES)

    def din(name, shape, dt):
        return nc.dram_tensor(name, shape, dt, kind="ExternalInput").ap()

    def dout(name, shape, dt):
        return nc.dram_tensor(name, shape, dt, kind="ExternalOutput").ap()

    xe8 = din("xe8", (128, ND, C), FP8)        # routed tokens, fp8 pairs
    w18 = din("w18", (128, ND, INTER), FP8)    # w1[e].T in DR pair layout
    w38 = din("w38", (128, ND, INTER), FP8)
    w28 = din("w28", (128, ND, DIM), FP8)      # w2[e].T in DR pair layout
    xs = din("xs", (128, ND, TS), BF16)        # shared-token slice
    ws1 = din("ws1", (128, ND, SHARED_INTER), BF16)
    ws3 = din("ws3", (128, ND, SHARED_INTER), BF16)
    ws2 = din("ws2", (128, NS, DIM), BF16)
    scb = din("scb", (128, 68), F32)  # packed biases + dequant scales
    ye = dout("ye", (ND, 128, C), BF16)
    ys = dout("ys", (ND, 128, TS), BF16)

    rblocks = _blocks(C)
    sblocks = _blocks(TS)

    with tile.TileContext(nc) as tc:
        with tc.tile_pool(name="const", bufs=1) as cpool, \
             tc.tile_pool(name="tmp", bufs=2) as tpool, \
             tc.tile_pool(name="hr", bufs=2) as hrpool, \
             tc.tile_pool(name="hsh", bufs=2) as hspool, \
             tc.tile_pool(name="yout", bufs=6) as ypool, \
             tc.tile_pool(name="ysout", bufs=2) as yspool, \
             tc.tile_pool(name="ps1p", bufs=3, space="PSUM") as ps1pool, \
             tc.tile_pool(name="ps3p", bufs=2, space="PSUM") as ps3pool, \
             tc.tile_pool(name="ps", bufs=3, space="PSUM") as pspool:

            # ---- PE warmup: ramp the DVFS clock while input DMA runs ----
            wsc = cpool.tile([128, 416], BF16, tag="warm")
            nc.vector.memset(wsc[:], 0.25)
            for _ in range(N_WARM):
                wps = pspool.tile([128, 416], F32, tag="psy",
                                  padded_shape=[128, BLK])
                nc.tensor.matmul(wps[:], wsc[:, 0:128], wsc[:],
                                 start=True, stop=True)

            # ---- input DMAs, large transfers in PE consumption order ----
            ball = cpool.tile([128, 68], F32, tag="scb")
            nc.sync.dma_start(ball[:], scb[:])
            b1c = lambda mi: ball[:, mi:mi + 1]
            b3c = lambda mi: ball[:, 8 + mi:9 + mi]
            b2c = lambda md: ball[:, 16 + md:17 + md]
            bs1c = lambda mi: ball[:, 24 + mi:25 + mi]
            bs3c = lambda mi: ball[:, 40 + mi:41 + mi]
            bs2c = lambda md: ball[:, 56 + md:57 + md]
            sc1 = ball[:, 64:65]
            sc3 = ball[:, 65:66]
            scy = ball[:, 66:67]

            hi = INTER // 2
            w1_t = cpool.tile([128, ND, INTER], FP8, tag="w1")
            w3_t = cpool.tile([128, ND, INTER], FP8, tag="w3")
            xe_t = cpool.tile([128, ND, C], FP8, tag="xe", name="xe")
            w2_t = cpool.tile([128, ND, DIM], FP8, tag="w2")
            off0, n0 = rblocks[0]
            nc.sync.dma_start(w1_t[:, :, 0:hi], w18[:, :, 0:hi])
            nc.sync.dma_start(xe_t[:, :, off0:off0 + n0],
                              xe8[:, :, off0:off0 + n0])
            nc.sync.dma_start(w3_t[:, :, 0:hi], w38[:, :, 0:hi])
            nc.sync.dma_start(w1_t[:, :, hi:INTER], w18[:, :, hi:INTER])
            nc.sync.dma_start(w3_t[:, :, hi:INTER], w38[:, :, hi:INTER])
            nc.sync.dma_start(w2_t[:], w28[:])
            for (off, n) in rblocks[1:]:
                nc.sync.dma_start(xe_t[:, :, off:off + n],
                                  xe8[:, :, off:off + n])
            xs_t = cpool.tile([128, ND, TS], BF16, tag="xs", name="xs")
            nc.sync.dma_start(xs_t[:], xs[:])
            ws1_t = cpool.tile([128, ND, SHARED_INTER], BF16, tag="ws1",
                               name="ws1")
            ws3_t = cpool.tile([128, ND, SHARED_INTER], BF16, tag="ws3",
                               name="ws3")
            ws2_t = cpool.tile([128, NS, DIM], BF16, tag="ws2", name="ws2")
            # shared-weight DMAs trickled across the routed block loop in
            # ~2MB chunks so routed output DMAs never queue behind them
            hsi = SHARED_INTER // 2
            ws_dmas = [
                (ws1_t[:, :, 0:hsi], ws1[:, :, 0:hsi]),
                (ws1_t[:, :, hsi:SHARED_INTER], ws1[:, :, hsi:SHARED_INTER]),
                (ws3_t[:, :, 0:hsi], ws3[:, :, 0:hsi]),
                (ws3_t[:, :, hsi:SHARED_INTER], ws3[:, :, hsi:SHARED_INTER]),
                (ws2_t[:, 0:NS // 2, :], ws2[:, 0:NS // 2, :]),
                (ws2_t[:, NS // 2:NS, :], ws2[:, NS // 2:NS, :]),
            ]
            nws = -(-len(ws_dmas) // len(rblocks))

            # ---- Phase 1: routed expert, fp8 DoubleRow ----
            h8s = {}

            def r_h8(bi):
                if bi not in h8s:
                    n = rblocks[bi][1]
                    h8s[bi] = [hrpool.tile([128, 2, n], FP8, tag=f"h8_{j}",
                                           name=f"h8{j}",
                                           padded_shape=[128, 2, BLK])
                               for j in range(NP)]
                return h8s[bi]

            def emit_mi(bi, mi):
                off, n = rblocks[bi]
                h8 = r_h8(bi)
                ps1 = ps1pool.tile([128, n], F32, tag="ps1",
                                   padded_shape=[128, BLK])
                for j in range(NP):
                    nc.tensor.matmul(
                        ps1[:], w1_t[:, 2 * j:2 * j + 2,
                                     mi * 128:(mi + 1) * 128],
                        xe_t[:, 2 * j:2 * j + 2, off:off + n],
                        start=(j == 0), stop=(j == NP - 1), perf_mode=DR)
                ps3 = ps3pool.tile([128, n], F32, tag="ps3",
                                   padded_shape=[128, BLK])
                for j in range(NP):
                    nc.tensor.matmul(
                        ps3[:], w3_t[:, 2 * j:2 * j + 2,
                                     mi * 128:(mi + 1) * 128],
                        xe_t[:, 2 * j:2 * j + 2, off:off + n],
                        start=(j == 0), stop=(j == NP - 1), perf_mode=DR)
                t1 = tpool.tile([128, n], BF16, tag="t1", name="t1",
                                padded_shape=[128, BLK])
                nc.scalar.activation(t1[:], ps1[:], SILU,
                                     bias=b1c(mi), scale=sc1)
                if general:
                    t3 = tpool.tile([128, n], BF16, tag="t3", name="t3",
                                    padded_shape=[128, BLK])
                    nc.vector.tensor_scalar(t3[:], ps3[:], sc3, b3c(mi),
                                            MUL, ADD)
                    nc.vector.tensor_mul(h8[mi // 2][:, mi % 2, :],
                                         t1[:], t3[:])
                else:
                    # s3 is chosen so ps3 == h-precursor * sh exactly
                    nc.vector.tensor_mul(h8[mi // 2][:, mi % 2, :],
                                         t1[:], ps3[:])

            for bi, (off, n) in enumerate(rblocks):
                for dst, src in ws_dmas[bi * nws:(bi + 1) * nws]:
                    nc.sync.dma_start(dst, src)
                for mi in (range(NI) if bi == 0 else range(2, NI)):
                    emit_mi(bi, mi)
                if bi + 1 < len(rblocks):
                    emit_mi(bi + 1, 0)  # lookahead covers h8 vector lag
                    emit_mi(bi + 1, 1)
                h8 = h8s.pop(bi)
                for md in range(ND):
                    psy = pspool.tile([128, n], F32, tag="psy",
                                      padded_shape=[128, BLK])
                    for j in range(NP):
                        nc.tensor.matmul(
                            psy[:], w2_t[:, 2 * j:2 * j + 2,
                                         md * 128:(md + 1) * 128],
                            h8[j][:, :, :],
                            start=(j == 0), stop=(j == NP - 1),
                            perf_mode=DR)
                    yt = ypool.tile([128, n], BF16, tag="yt", name="yt",
                                    padded_shape=[128, BLK])
                    nc.scalar.activation(yt[:], psy[:], IDENT,
                                         bias=b2c(md), scale=scy)
                    nc.sync.dma_start(ye[md][:, off:off + n], yt[:])

            # ---- Phase 2: shared expert, bf16 ----
            for (off, n) in sblocks:
                hs = [hspool.tile([128, n], BF16, tag=f"hs_{mi}",
                                  name=f"hs{mi}", padded_shape=[128, BLK])
                      for mi in range(NS)]
                for mi in range(NS):
                    ps1 = ps1pool.tile([128, n], F32, tag="ps1",
                                       padded_shape=[128, BLK])
                    for dk in range(ND):
                        nc.tensor.matmul(
                            ps1[:], ws1_t[:, dk, mi * 128:(mi + 1) * 128],
                            xs_t[:, dk, off:off + n],
                            start=(dk == 0), stop=(dk == ND - 1))
                    ps3 = ps3pool.tile([128, n], F32, tag="ps3",
                                       padded_shape=[128, BLK])
                    for dk in range(ND):
                        nc.tensor.matmul(
                            ps3[:], ws3_t[:, dk, mi * 128:(mi + 1) * 128],
                            xs_t[:, dk, off:off + n],
                            start=(dk == 0), stop=(dk == ND - 1))
                    t1 = tpool.tile([128, n], BF16, tag="t1", name="t1",
                                    padded_shape=[128, BLK])
                    nc.scalar.activation(t1[:], ps1[:], SILU, bias=bs1c(mi))
                    if general:
                        t3 = tpool.tile([128, n], BF16, tag="t3", name="t3",
                                        padded_shape=[128, BLK])
                        nc.vector.tensor_scalar(t3[:], ps3[:], 1.0, bs3c(mi),
                                                MUL, ADD)
                        nc.vector.tensor_mul(hs[mi][:], t1[:], t3[:])
                    else:
                        nc.vector.tensor_mul(hs[mi][:], t1[:], ps3[:])
                for md in range(ND):
                    psy = pspool.tile([128, n], F32, tag="psy",
                                      padded_shape=[128, BLK])
                    for mi in range(NS):
                        nc.tensor.matmul(
                            psy[:], ws2_t[:, mi, md * 128:(md + 1) * 128],
                            hs[mi][:],
                            start=(mi == 0), stop=(mi == NS - 1))
                    yts = yspool.tile([128, n], BF16, tag="yts", name="yts",
                                      padded_shape=[128, BLK])
                    nc.scalar.activation(yts[:], psy[:], IDENT, bias=bs2c(md))
                    nc.sync.dma_start(ys[md][:, off:off + n], yts[:])

    nc.compile()
    return nc


def _q8(a):
    return np.clip(a, -448.0, 448.0).astype(E4NP)


def _pack_w(w, scale):
    """[out, K] weight -> [128, K/128, out] fp8 DoubleRow pair layout
    (partition-major; k-subtile pairs adjacent in the middle dim)."""
    K = w.shape[1]
    A = (w.T * scale).reshape(K // 256, 2, 128, w.shape[0])
    return _q8(np.ascontiguousarray(
        A.transpose(2, 0, 1, 3).reshape(128, K // 128, w.shape[0])))


def _pack_x(xg, scale, C):
    """[n, DIM] tokens -> [128, DIM/128, C] fp8 DoubleRow pair layout."""
    A = np.zeros((DIM, C), np.float32)
    A[:, :xg.shape[0]] = (xg * scale).T
    A = A.reshape(NP, 2, 128, C)
    return _q8(np.ascontiguousarray(
        A.transpose(2, 0, 1, 3).reshape(128, ND, C)))


def _pack_bf(w_t, nk):
    """[K, M] (already transposed) -> [128, nk, M] bf16."""
    K, M = w_t.shape
    return np.ascontiguousarray(
        w_t.reshape(nk, 128, M).transpose(1, 0, 2)).astype(BFNP)


def _gate_host(xt, gate_w, gate_b):
    logits = xt.astype(np.float64) @ gate_w.astype(np.float64).T \
        + gate_b.astype(np.float64)
    m = logits.max(axis=-1, keepdims=True)
    p = np.exp(logits - m)
    scores = p / p.sum(axis=-1, keepdims=True)
    order = np.argsort(-scores, axis=1, kind="stable")
    top_i = order[:, :TOPK]
    top_w = (np.take_along_axis(scores, top_i, axis=1)
             * ROUTE_SCALE).astype(np.float32)
    return top_i, top_w


def run(inputs, trace=False):
    x = np.ascontiguousarray(np.asarray(inputs["x"], dtype=np.float32))
    gate_w = np.asarray(inputs["gate_w"], dtype=np.float32)
    gate_b = np.asarray(inputs["gate_b"], dtype=np.float32)
    w1 = np.asarray(inputs["w1"], dtype=np.float32)
    b1 = np.asarray(inputs["b1"], dtype=np.float32)
    w3 = np.asarray(inputs["w3"], dtype=np.float32)
    b3 = np.asarray(inputs["b3"], dtype=np.float32)
    w2 = np.asarray(inputs["w2"], dtype=np.float32)
    b2 = np.asarray(inputs["b2"], dtype=np.float32)
    ws1 = np.asarray(inputs["ws1"], dtype=np.float32)
    bs1 = np.asarray(inputs["bs1"], dtype=np.float32)
    ws3 = np.asarray(inputs["ws3"], dtype=np.float32)
    bs3 = np.asarray(inputs["bs3"], dtype=np.float32)
    ws2 = np.asarray(inputs["ws2"], dtype=np.float32)
    bs2 = np.asarray(inputs["bs2"], dtype=np.float32)

    xt = x.reshape(T, DIM)
    top_i, top_w = _gate_host(xt, gate_w, gate_b)

    idx, wgt = [], []
    for e in range(E):
        toks = np.nonzero((top_i == e).any(axis=1))[0]
        idx.append(toks)
        slot = (top_i[toks] == e)
        wgt.append(top_w[toks][slot])

    cmax = max(len(i) for i in idx)
    C = max(256, -(-cmax // 32) * 32)

    # fp8 scales: per-tensor for x, per-expert per-tensor for weights; the
    # h scale comes from a 32-token fp32 sample of the true h distribution.
    sx = 8.0 / max(xt.std(), 1e-30)
    xprobe = xt[:32]
    s1 = np.empty(E, np.float64); s3 = np.empty(E, np.float64)
    s2 = np.empty(E, np.float64); sh = np.empty(E, np.float64)
    for e in range(E):
        s1[e] = 16.0 / max(w1[e].std(), 1e-30)
        s2[e] = 16.0 / max(w2[e].std(), 1e-30)
        a = xprobe @ w1[e].T + b1[e]
        bb = xprobe @ w3[e].T + b3[e]
        h = a / (1.0 + np.exp(-a)) * bb
        # tie s3 to the h scale so ps3 needs no dequant (sh == sx*s3)
        s3[e] = 8.0 / max(h.std(), 1e-30) / sx
        sh[e] = sx * s3[e]

    ws1p = _pack_bf(ws1.T, ND)
    ws3p = _pack_bf(ws3.T, ND)
    ws2p = _pack_bf(ws2.T, NS)

    in_maps = []
    for e in range(E):
        scbuf = np.zeros((128, 68), np.float32)
        scbuf[:, 0:8] = b1[e].reshape(8, 128).T
        scbuf[:, 8:16] = (b3[e] * sh[e]).reshape(8, 128).T
        scbuf[:, 16:24] = b2[e].reshape(8, 128).T
        scbuf[:, 24:40] = bs1.reshape(16, 128).T
        scbuf[:, 40:56] = bs3.reshape(16, 128).T
        scbuf[:, 56:64] = bs2.reshape(8, 128).T
        scbuf[:, 64] = 1.0 / (sx * s1[e])
        scbuf[:, 65] = sh[e] / (sx * s3[e])
        scbuf[:, 66] = 1.0 / (sh[e] * s2[e])
        sl = slice(TS * e, TS * (e + 1))
        in_maps.append({
            "xe8": _pack_x(xt[idx[e]], sx, C),
            "w18": _pack_w(w1[e], s1[e]),
            "w38": _pack_w(w3[e], s3[e]),
            "w28": _pack_w(w2[e], s2[e]),
            "xs": _pack_bf(xt[sl].T, ND),
            "ws1": ws1p, "ws3": ws3p, "ws2": ws2p,
            "scb": scbuf,
        })

    general = bool(np.any(b3) or np.any(bs3))
    key = (C, general)
    if key not in _program_cache:
        _program_cache[key] = build_program(C, general)
    nc = _program_cache[key]

    res = bass_utils.run_bass_kernel_spmd(
        nc, in_maps, core_ids=list(range(N_CORES)), trace=trace)

    y = np.empty((T, DIM), np.float32)
    for e in range(E):
        sl = slice(TS * e, TS * (e + 1))
        y[sl] = res.results[e]["ys"].reshape(DIM, TS).T.astype(np.float32)
    for e in range(E):
        yee = res.results[e]["ye"].reshape(DIM, C).astype(np.float32)
        y[idx[e]] += yee[:, :len(idx[e])].T * wgt[e][:, None]
    return y.reshape(B, S, DIM), res


def kernel(**inputs) -> np.ndarray:
    out, _ = run(inputs, trace=False)
    return out


# revision 19
# speedup vs baseline: 1.0146x; 1.0080x over previous
"""MoE (DeepSeek-style) routed+shared expert forward on 8 TRN2 NeuronCores.

Strategy (expert-parallel, host-side dispatch):
  - Host computes the gate (softmax + top-2) in float64 and gathers each
    expert's routed tokens; core e processes expert e's tokens (padded to
    capacity C) plus a 1/8 slice of all tokens through the replicated
    shared-expert MLP.
  - Routed expert runs in fp8(e4m3) with DoubleRow matmuls (2x PE rate).
    Host quantizes x and the expert weights with per-tensor scales; the
    w1-path dequant folds into the SILU activation's scale operand and
    the w3 scale is tied to the sampled h distribution (s3 = sh/sx) so
    the ps3 PSUM result IS h*sh exactly — the h tile is produced by a
    single vector multiply reading PSUM, no dequant op. Error budget:
    the routed path carries only ~23% of the output norm (gate weights
    are softmax scores ~0.2), so fp8's ~6% relative error lands at
    ~1.6e-2 overall, within the 2e-2 tolerance.
  - Shared expert (97% of the output norm) stays bf16.
  - All weights are SBUF-resident; inputs arrive via a handful of large
    DMAs (one per operand) ordered to match PE consumption, with the
    shared-expert weights trickled in across the routed block loop so
    routed output DMAs never sit behind a multi-MB preload backlog.
  - Warmup matmuls on scratch SBUF run during the input DMA so the PE's
    DVFS clock is fully ramped when real work arrives.
"""

import sys

if "/opt/trn_rl_repo" not in sys.path:
    sys.path.insert(0, "/opt/trn_rl_repo")

import ml_dtypes
import numpy as np

import concourse.bass as bass
import concourse.tile as tile
from concourse import bacc, mybir
from concourse import bass_utils
from concourse.alu_op_type import AluOpType

B, S, DIM = 4, 2048, 1024
T = B * S
INTER = 1024
E = 8
TOPK = 2
ROUTE_SCALE = 1.0
SHARED_INTER = 2048
N_CORES = 8
TS = T // N_CORES  # shared-expert tokens per core
BLK = 512
N_WARM = 17

F32 = mybir.dt.float32
BF16 = mybir.dt.bfloat16
FP8 = mybir.dt.float8e4
SILU = mybir.ActivationFunctionType.Silu
IDENT = mybir.ActivationFunctionType.Identity
DR = mybir.MatmulPerfMode.DoubleRow
MUL = AluOpType.mult
ADD = AluOpType.add

E4NP = ml_dtypes.float8_e4m3fn
BFNP = ml_dtypes.bfloat16

ND = DIM // 128           # 8 k-tiles over DIM
NP = ND // 2              # 4 DoubleRow k-pair tiles over DIM
NI = INTER // 128         # 8 tiles over INTER
NS = SHARED_INTER // 128  # 16 tiles over SHARED_INTER

_program_cache = {}


def _blocks(total):
    """Split into <=512-wide even blocks of near-equal size (all >=256 so
    per-instruction LDWEIGHTS overhead stays hidden)."""
    nb = -(-total // BLK)
    b = -(-total // (nb * 32)) * 32
    sizes = [b] * (nb - 1) + [total - b * (nb - 1)]
    assert all(256 <= s <= BLK and s % 2 == 0 for s in sizes), sizes
    out, o = [], 0
    for s in sizes:
        out.append((o, s))
        o += s
    return out


def build_program(C, general):
    nc = bacc.Bacc("TRN2", target_bir_lowering=False, debug=False,
                   num_devices=N_CORES)

    def din(name, shape, dt):
        return nc.dram_tensor(name, shape, dt, kind="ExternalInput").ap()

    def dout(name, shape, dt):
        return nc.dram_tensor(name, shape, dt, kind="ExternalOutput").ap()

    n0r = _blocks(C)[0][1]
    xe8a = din("xe8a", (128, ND, n0r), FP8)    # routed tokens, block 0
    xe8b = din("xe8b", (128, ND, C - n0r), FP8)
    w18 = din("w18", (2, 128, ND, INTER // 2), FP8)  # w1[e].T, DR pairs,
    w38 = din("w38", (2, 128, ND, INTER // 2), FP8)  # contiguous halves
    w28 = din("w28", (128, ND, DIM), FP8)      # w2[e].T in DR pair layout
    xs = din("xs", (128, ND, TS), BF16)        # shared-token slice
    ws1 = din("ws1", (128, ND, SHARED_INTER), BF16)
    ws3 = din("ws3", (128, ND, SHARED_INTER), BF16)
    ws2 = din("ws2", (128, NS, DIM), BF16)
    scb = din("scb", (128, 68), F32)  # packed biases + dequant scales
    ye = dout("ye", (ND, 128, C), BF16)
    ys = dout("ys", (ND, 128, TS), BF16)

    rblocks = _blocks(C)
    sblocks = _blocks(TS)

    with tile.TileContext(nc) as tc:
        with tc.tile_pool(name="const", bufs=1) as cpool, \
             tc.tile_pool(name="tmp", bufs=2) as tpool, \
             tc.tile_pool(name="hr", bufs=2) as hrpool, \
             tc.tile_pool(name="hsh", bufs=2) as hspool, \
             tc.tile_pool(name="yout", bufs=6) as ypool, \
             tc.tile_pool(name="ysout", bufs=2) as yspool, \
             tc.tile_pool(name="ps1p", bufs=3, space="PSUM") as ps1pool, \
             tc.tile_pool(name="ps3p", bufs=2, space="PSUM") as ps3pool, \
             tc.tile_pool(name="ps", bufs=3, space="PSUM") as pspool:

            # ---- PE warmup: ramp the DVFS clock while input DMA runs ----
            wsc = cpool.tile([128, 416], BF16, tag="warm")
            nc.vector.memset(wsc[:], 0.25)
            for _ in range(N_WARM):
                wps = pspool.tile([128, 416], F32, tag="psy",
                                  padded_shape=[128, BLK])
                nc.tensor.matmul(wps[:], wsc[:, 0:128], wsc[:],
                                 start=True, stop=True)

            # ---- input DMAs, large transfers in PE consumption order ----
            ball = cpool.tile([128, 68], F32, tag="scb")
            nc.sync.dma_start(ball[:], scb[:])
            b1c = lambda mi: ball[:, mi:mi + 1]
            b3c = lambda mi: ball[:, 8 + mi:9 + mi]
            b2c = lambda md: ball[:, 16 + md:17 + md]
            bs1c = lambda mi: ball[:, 24 + mi:25 + mi]
            bs3c = lambda mi: ball[:, 40 + mi:41 + mi]
            bs2c = lambda md: ball[:, 56 + md:57 + md]
            sc1 = ball[:, 64:65]
            sc3 = ball[:, 65:66]
            scy = ball[:, 66:67]

            hi = INTER // 2
            w1_t = cpool.tile([128, ND, INTER], FP8, tag="w1")
            w3_t = cpool.tile([128, ND, INTER], FP8, tag="w3")
            xe_t = cpool.tile([128, ND, C], FP8, tag="xe", name="xe")
            w2_t = cpool.tile([128, ND, DIM], FP8, tag="w2")
            off0, n0 = rblocks[0]
            nc.sync.dma_start(w1_t[:, :, 0:hi], w18[0])
            nc.sync.dma_start(xe_t[:, :, off0:off0 + n0], xe8a[:])
            nc.sync.dma_start(w3_t[:, :, 0:hi], w38[0])
            nc.sync.dma_start(w1_t[:, :, hi:INTER], w18[1])
            nc.sync.dma_start(w3_t[:, :, hi:INTER], w38[1])
            nc.sync.dma_start(w2_t[:], w28[:])
            for (off, n) in rblocks[1:]:
                nc.sync.dma_start(xe_t[:, :, off:off + n],
                                  xe8b[:, :, off - n0:off - n0 + n])
            xs_t = cpool.tile([128, ND, TS], BF16, tag="xs", name="xs")
            nc.sync.dma_start(xs_t[:], xs[:])
            ws1_t = cpool.tile([128, ND, SHARED_INTER], BF16, tag="ws1",
                               name="ws1")
            ws3_t = cpool.tile([128, ND, SHARED_INTER], BF16, tag="ws3",
                               name="ws3")
            ws2_t = cpool.tile([128, NS, DIM], BF16, tag="ws2", name="ws2")
            # shared-weight DMAs trickled across the routed block loop in
            # ~2MB chunks so routed output DMAs never queue behind them
            hsi = SHARED_INTER // 2
            ws_dmas = [
                (ws1_t[:, :, 0:hsi], ws1[:, :, 0:hsi]),
                (ws1_t[:, :, hsi:SHARED_INTER], ws1[:, :, hsi:SHARED_INTER]),
                (ws3_t[:, :, 0:hsi], ws3[:, :, 0:hsi]),
                (ws3_t[:, :, hsi:SHARED_INTER], ws3[:, :, hsi:SHARED_INTER]),
                (ws2_t[:, 0:NS // 2, :], ws2[:, 0:NS // 2, :]),
                (ws2_t[:, NS // 2:NS, :], ws2[:, NS // 2:NS, :]),
            ]
            nws = -(-len(ws_dmas) // len(rblocks))

            # ---- Phase 1: routed expert, fp8 DoubleRow ----
            h8s = {}

            def r_h8(bi):
                if bi not in h8s:
                    n = rblocks[bi][1]
                    h8s[bi] = [hrpool.tile([128, 2, n], FP8, tag=f"h8_{j}",
                                           name=f"h8{j}",
                                           padded_shape=[128, 2, BLK])
                               for j in range(NP)]
                return h8s[bi]

            def emit_mi(bi, mi):
                off, n = rblocks[bi]
                h8 = r_h8(bi)
                ps1 = ps1pool.tile([128, n], F32, tag="ps1",
                                   padded_shape=[128, BLK])
                for j in range(NP):
                    nc.tensor.matmul(
                        ps1[:], w1_t[:, 2 * j:2 * j + 2,
                                     mi * 128:(mi + 1) * 128],
                        xe_t[:, 2 * j:2 * j + 2, off:off + n],
                        start=(j == 0), stop=(j == NP - 1), perf_mode=DR)
                ps3 = ps3pool.tile([128, n], F32, tag="ps3",
                                   padded_shape=[128, BLK])
                for j in range(NP):
                    nc.tensor.matmul(
                        ps3[:], w3_t[:, 2 * j:2 * j + 2,
                                     mi * 128:(mi + 1) * 128],
                        xe_t[:, 2 * j:2 * j + 2, off:off + n],
                        start=(j == 0), stop=(j == NP - 1), perf_mode=DR)
                t1 = tpool.tile([128, n], BF16, tag="t1", name="t1",
                                padded_shape=[128, BLK])
                nc.scalar.activation(t1[:], ps1[:], SILU,
                                     bias=b1c(mi), scale=sc1)
                if general:
                    t3 = tpool.tile([128, n], BF16, tag="t3", name="t3",
                                    padded_shape=[128, BLK])
                    nc.vector.tensor_scalar(t3[:], ps3[:], sc3, b3c(mi),
                                            MUL, ADD)
                    nc.vector.tensor_mul(h8[mi // 2][:, mi % 2, :],
                                         t1[:], t3[:])
                else:
                    # s3 is chosen so ps3 == h-precursor * sh exactly
                    nc.vector.tensor_mul(h8[mi // 2][:, mi % 2, :],
                                         t1[:], ps3[:])

            for bi, (off, n) in enumerate(rblocks):
                for dst, src in ws_dmas[bi * nws:(bi + 1) * nws]:
                    nc.sync.dma_start(dst, src)
                for mi in (range(NI) if bi == 0 else range(2, NI)):
                    emit_mi(bi, mi)
                if bi + 1 < len(rblocks):
                    emit_mi(bi + 1, 0)  # lookahead covers h8 vector lag
                    emit_mi(bi + 1, 1)
                h8 = h8s.pop(bi)
                for md in range(ND):
                    psy = pspool.tile([128, n], F32, tag="psy",
                                      padded_shape=[128, BLK])
                    for j in range(NP):
                        nc.tensor.matmul(
                            psy[:], w2_t[:, 2 * j:2 * j + 2,
                                         md * 128:(md + 1) * 128],
                            h8[j][:, :, :],
                            start=(j == 0), stop=(j == NP - 1),
                            perf_mode=DR)
                    yt = ypool.tile([128, n], BF16, tag="yt", name="yt",
                                    padded_shape=[128, BLK])
                    nc.scalar.activation(yt[:], psy[:], IDENT,
                                         bias=b2c(md), scale=scy)
                    nc.sync.dma_start(ye[md][:, off:off + n], yt[:])

            # ---- Phase 2: shared expert, bf16 ----
            for (off, n) in sblocks:
                hs = [hspool.tile([128, n], BF16, tag=f"hs_{mi}",
                                  name=f"hs{mi}", padded_shape=[128, BLK])
                      for mi in range(NS)]
                for mi in range(NS):
                    ps1 = ps1pool.tile([128, n], F32, tag="ps1",
                                       padded_shape=[128, BLK])
                    for dk in range(ND):
                        nc.tensor.matmul(
                            ps1[:], ws1_t[:, dk, mi * 128:(mi + 1) * 128],
                            xs_t[:, dk, off:off + n],
                            start=(dk == 0), stop=(dk == ND - 1))
                    ps3 = ps3pool.tile([128, n], F32, tag="ps3",
                                       padded_shape=[128, BLK])
                    for dk in range(ND):
                        nc.tensor.matmul(
                            ps3[:], ws3_t[:, dk, mi * 128:(mi + 1) * 128],
                            xs_t[:, dk, off:off + n],
                            start=(dk == 0), stop=(dk == ND - 1))
                    t1 = tpool.tile([128, n], BF16, tag="t1", name="t1",
                                    padded_shape=[128, BLK])
                    nc.scalar.activation(t1[:], ps1[:], SILU, bias=bs1c(mi))
                    if general:
                        t3 = tpool.tile([128, n], BF16, tag="t3", name="t3",
                                        padded_shape=[128, BLK])
                        nc.vector.tensor_scalar(t3[:], ps3[:], 1.0, bs3c(mi),
                                                MUL, ADD)
                        nc.vector.tensor_mul(hs[mi][:], t1[:], t3[:])
                    else:
                        nc.vector.tensor_mul(hs[mi][:], t1[:], ps3[:])
                for md in range(ND):
                    psy = pspool.tile([128, n], F32, tag="psy",
                                      padded_shape=[128, BLK])
                    for mi in range(NS):
                        nc.tensor.matmul(
                            psy[:], ws2_t[:, mi, md * 128:(md + 1) * 128],
                            hs[mi][:],
                            start=(mi == 0), stop=(mi == NS - 1))
                    yts = yspool.tile([128, n], BF16, tag="yts", name="yts",
                                      padded_shape=[128, BLK])
                    nc.scalar.activation(yts[:], psy[:], IDENT, bias=bs2c(md))
                    nc.sync.dma_start(ys[md][:, off:off + n], yts[:])

    nc.compile()
    return nc


def _q8(a):
    return np.clip(a, -448.0, 448.0).astype(E4NP)


def _pack_w(w, scale):
    """[out, K] weight -> [128, K/128, out] fp8 DoubleRow pair layout
    (partition-major; k-subtile pairs adjacent in the middle dim)."""
    K = w.shape[1]
    A = (w.T * scale).reshape(K // 256, 2, 128, w.shape[0])
    return _q8(np.ascontiguousarray(
        A.transpose(2, 0, 1, 3).reshape(128, K // 128, w.shape[0])))


def _pack_x(xg, scale, C):
    """[n, DIM] tokens -> [128, DIM/128, C] fp8 DoubleRow pair layout."""
    A = np.zeros((DIM, C), np.float32)
    A[:, :xg.shape[0]] = (xg * scale).T
    A = A.reshape(NP, 2, 128, C)
    return _q8(np.ascontiguousarray(
        A.transpose(2, 0, 1, 3).reshape(128, ND, C)))


def _pack_bf(w_t, nk):
    """[K, M] (already transposed) -> [128, nk, M] bf16."""
    K, M = w_t.shape
    return np.ascontiguousarray(
        w_t.reshape(nk, 128, M).transpose(1, 0, 2)).astype(BFNP)


def _gate_host(xt, gate_w, gate_b):
    logits = xt.astype(np.float64) @ gate_w.astype(np.float64).T \
        + gate_b.astype(np.float64)
    m = logits.max(axis=-1, keepdims=True)
    p = np.exp(logits - m)
    scores = p / p.sum(axis=-1, keepdims=True)
    order = np.argsort(-scores, axis=1, kind="stable")
    top_i = order[:, :TOPK]
    top_w = (np.take_along_axis(scores, top_i, axis=1)
             * ROUTE_SCALE).astype(np.float32)
    return top_i, top_w


def run(inputs, trace=False):
    x = np.ascontiguousarray(np.asarray(inputs["x"], dtype=np.float32))
    gate_w = np.asarray(inputs["gate_w"], dtype=np.float32)
    gate_b = np.asarray(inputs["gate_b"], dtype=np.float32)
    w1 = np.asarray(inputs["w1"], dtype=np.float32)
    b1 = np.asarray(inputs["b1"], dtype=np.float32)
    w3 = np.asarray(inputs["w3"], dtype=np.float32)
    b3 = np.asarray(inputs["b3"], dtype=np.float32)
    w2 = np.asarray(inputs["w2"], dtype=np.float32)
    b2 = np.asarray(inputs["b2"], dtype=np.float32)
    ws1 = np.asarray(inputs["ws1"], dtype=np.float32)
    bs1 = np.asarray(inputs["bs1"], dtype=np.float32)
    ws3 = np.asarray(inputs["ws3"], dtype=np.float32)
    bs3 = np.asarray(inputs["bs3"], dtype=np.float32)
    ws2 = np.asarray(inputs["ws2"], dtype=np.float32)
    bs2 = np.asarray(inputs["bs2"], dtype=np.float32)

    xt = x.reshape(T, DIM)
    top_i, top_w = _gate_host(xt, gate_w, gate_b)

    idx, wgt = [], []
    for e in range(E):
        toks = np.nonzero((top_i == e).any(axis=1))[0]
        idx.append(toks)
        slot = (top_i[toks] == e)
        wgt.append(top_w[toks][slot])

    cmax = max(len(i) for i in idx)
    C = max(256, -(-cmax // 32) * 32)

    # fp8 scales: per-tensor for x, per-expert per-tensor for weights; the
    # h scale comes from a 32-token fp32 sample of the true h distribution.
    sx = 8.0 / max(xt.std(), 1e-30)
    xprobe = xt[:32]
    s1 = np.empty(E, np.float64); s3 = np.empty(E, np.float64)
    s2 = np.empty(E, np.float64); sh = np.empty(E, np.float64)
    for e in range(E):
        s1[e] = 16.0 / max(w1[e].std(), 1e-30)
        s2[e] = 16.0 / max(w2[e].std(), 1e-30)
        a = xprobe @ w1[e].T + b1[e]
        bb = xprobe @ w3[e].T + b3[e]
        h = a / (1.0 + np.exp(-a)) * bb
        # tie s3 to the h scale so ps3 needs no dequant (sh == sx*s3)
        s3[e] = 8.0 / max(h.std(), 1e-30) / sx
        sh[e] = sx * s3[e]

    ws1p = _pack_bf(ws1.T, ND)
    ws3p = _pack_bf(ws3.T, ND)
    ws2p = _pack_bf(ws2.T, NS)

    in_maps = []
    for e in range(E):
        scbuf = np.zeros((128, 68), np.float32)
        scbuf[:, 0:8] = b1[e].reshape(8, 128).T
        scbuf[:, 8:16] = (b3[e] * sh[e]).reshape(8, 128).T
        scbuf[:, 16:24] = b2[e].reshape(8, 128).T
        scbuf[:, 24:40] = bs1.reshape(16, 128).T
        scbuf[:, 40:56] = bs3.reshape(16, 128).T
        scbuf[:, 56:64] = bs2.reshape(8, 128).T
        scbuf[:, 64] = 1.0 / (sx * s1[e])
        scbuf[:, 65] = sh[e] / (sx * s3[e])
        scbuf[:, 66] = 1.0 / (sh[e] * s2[e])
        sl = slice(TS * e, TS * (e + 1))
        xep = _pack_x(xt[idx[e]], sx, C)
        w1p = _pack_w(w1[e], s1[e])
        w3p = _pack_w(w3[e], s3[e])
        n0r = _blocks(C)[0][1]
        in_maps.append({
            "xe8a": np.ascontiguousarray(xep[:, :, :n0r]),
            "xe8b": np.ascontiguousarray(xep[:, :, n0r:]),
            "w18": np.ascontiguousarray(
                np.stack([w1p[:, :, :INTER // 2], w1p[:, :, INTER // 2:]])),
            "w38": np.ascontiguousarray(
                np.stack([w3p[:, :, :INTER // 2], w3p[:, :, INTER // 2:]])),
            "w28": _pack_w(w2[e], s2[e]),
            "xs": _pack_bf(xt[sl].T, ND),
            "ws1": ws1p, "ws3": ws3p, "ws2": ws2p,
            "scb": scbuf,
        })

    general = bool(np.any(b3) or np.any(bs3))
    key = (C, general)
    if key not in _program_cache:
        _program_cache[key] = build_program(C, general)
    nc = _program_cache[key]

    res = bass_utils.run_bass_kernel_spmd(
        nc, in_maps, core_ids=list(range(N_CORES)), trace=trace)

    y = np.empty((T, DIM), np.float32)
    for e in range(E):
        sl = slice(TS * e, TS * (e + 1))
        y[sl] = res.results[e]["ys"].reshape(DIM, TS).T.astype(np.float32)
    for e in range(E):
        yee = res.results[e]["ye"].reshape(DIM, C).astype(np.float32)
        y[idx[e]] += yee[:, :len(idx[e])].T * wgt[e][:, None]
    return y.reshape(B, S, DIM), res


def kernel(**inputs) -> np.ndarray:
    out, _ = run(inputs, trace=False)
    return out
